# revision 1
# baseline (speedup 1.0000x reference)
"""Morphological dilation (depthwise 3x3, additive SE) on 8 TRN2 NeuronCores.

out[b,c,h,w] = max_{dy,dx in {-1,0,1}} ( x[b,c,h+dy,w+dx] + k[c, (dy+1)*3+(dx+1)] )
with zero padding outside the image.

Sharding: batch -> 8 cores (1 image each). Per core, partitions = (h_half, c)
(2*64 = 128), free dim = rows x cols, processed in row chunks.

The entire 9-term max reduction runs as EIGHT fused custom-DVE ops
(ADD_MAX_ANT: out = max(in0 + s0, in1), hand-written 2x_1p uop program, so it
matches tensor_tensor's 2-elem/cycle fp16 throughput while folding the
per-channel kernel constant in for free):

  - xe     = zero-padded input, fp16, [128, 114, 226] (halo rows + cols); the
             six dx=+-1 terms read it at 4B-aligned column offsets 0 / 2.
  - x2     = host-precomputed xpad(col +1) + k4, fp16, [128, 114, 224]; its
             row-0 view seeds the chain (term T4 free), and the two remaining
             dx=0 terms derive from it with delta constants k1-k4 / k7-k4
             (aligned row-shifted reads; a raw odd-column read of xe would
             drop the op to 1x mode).
  - chain: o = AM(xe(dy,dx), k_i, [x2 seed | o]) x6, then o = AM(x2(dy), dk, o) x2.

No ScalarE / tensor_scalar / GpSimd compute at all: DVE runs only 2x_1p ops
(never grabbing the shared 2-port pair), so the GpSimd SWDGE output DMAs
never contend. Input loads are single 128-partition dma_starts (all 16 DMA
ports) on the sync HWDGE queue.
"""

import numpy as np

_CACHE = {}

C = 64
H = 224
W = 224
HALF = 112
ROWS = HALF + 2  # per-half rows incl. 1-row halo each side
CHUNKS = (8, 12, 24, 28, 28, 12)

_ADDMAX_NAME = "ADD_MAX_ANT"


def _register_addmax():
    """Register the fused 2x add-max custom DVE op (idempotent)."""
    from concourse import dve_ops
    from concourse.dve_spec import Spec, Src0, Src1, C0, maxx, lower
    from concourse.dve_uop import (
        AluInp,
        AluOp,
        DelayInp,
        DveOpSpec,
        InpSel,
        OutPath,
        OutSel,
        Trigger,
        UopConfig,
    )

    if _ADDMAX_NAME in dve_ops._SUB_OPCODE_FOR_NAME:
        return next(op for op in dve_ops.OPS if op.name == _ADDMAX_NAME)

    def _ref(in0, in1, s0, s1, imm2):
        return np.maximum(
            in0.astype(np.float32) + s0, in1.astype(np.float32)
        ).astype(np.float32)

    spec = Spec(body=maxx(Src0 + C0, Src1), reference=_ref)

    def _build_2x():
        """Mirror of stock tensor_tensor's 2x_1p program (opcode-table slot 9)
        with the single INSTRUCTION_OP stage split into concrete ADD + MAX.

        Input lanes: 0=SRC_0, 1=SRC_1, 2=SRC_0_HI, 3=SRC_1_HI, 4=CONST_0.
        At blk0: lane0 -> PREV_ALU_OUT, lane(k+1) -> PREV_DELAY_k.
        """
        u = UopConfig()
        u.enable_input(InpSel.SRC_0, 0)
        u.enable_input(InpSel.SRC_1, 1)
        u.enable_input(InpSel.SRC_0_HI, 2)
        u.enable_input(InpSel.SRC_1_HI, 3)
        u.enable_input(InpSel.CONST_0, 4)
        u.require_inp0 = 1
        u.require_inp1 = 1
        u.trigger = (Trigger.SRC_TENSOR_DONE, Trigger.NONE, Trigger.NONE)

        dp = u.datapath_config
        # blk0: a0 = SRC_0 + CONST_0 ; carry SRC_1, SRC_0_HI, SRC_1_HI, CONST_0
        dp[0].enable_alu(AluOp.ADD, AluInp.PREV_ALU_OUT, AluInp.PREV_DELAY_3)
        dp[0].pass_through_delay(0, 1, 2, 3)
        # blk1: r0 = max(a0, SRC_1)
        dp[1].enable_alu(AluOp.MAX, AluInp.PREV_ALU_OUT, AluInp.PREV_DELAY_0)
        dp[1].pass_through_delay(1, 2, 3)
        # blk2: a1 = SRC_0_HI + CONST_0 ; d0 <- r0
        dp[2].enable_alu(AluOp.ADD, AluInp.PREV_DELAY_1, AluInp.PREV_DELAY_3)
        dp[2].enable_delay_from_src(DelayInp.PREV_ALU_OUT, 0)
        dp[2].pass_through_delay(2)
        # blk3: r1 = max(a1, SRC_1_HI) ; carry r0
        dp[3].enable_alu(AluOp.MAX, AluInp.PREV_ALU_OUT, AluInp.PREV_DELAY_2)
        dp[3].pass_through_delay(0)
        # blk4: alu <- r0, d0 <- r1 (swap, as stock does)
        dp[4].enable_alu(AluOp.BYPASS, AluInp.PREV_DELAY_0, AluInp.PREV_DELAY_0)
        dp[4].enable_delay_from_src(DelayInp.PREV_ALU_OUT, 0)
        # blk5..7: bypass r0 down the alu pipe, carry r1
        for b in range(5, 8):
            dp[b].pass_through_alu()
            dp[b].pass_through_delay(0)

        u.enable_output(OutSel.ALU_OUT, OutPath.WR0_LO)
        u.enable_output(OutSel.DELAY_0, OutPath.WR0_HI)
        return u

    class _AddMaxOp:
        name = _ADDMAX_NAME
        subdim = False
        perf_en = {}
        uops_sha = {}

        def __init__(self):
            self.spec = spec
            self._cache = {}

        def compile(self, ver):
            if ver in self._cache:
                return self._cache[ver]
            assert ver == "v3", "ADD_MAX_ANT 2x program authored for TRN2/v3"
            s = DveOpSpec(
                name=self.name,
                opcode=dve_ops.get_dve_sub_opcode(self.name),
                uops=lower(self.spec, ver=ver),
                uops_2x=[_build_2x()],
                rd1_en=True,
                perf_max=1,
            )
            s.validate(ver)
            self._cache[ver] = s
            return s

    op = _AddMaxOp()
    dve_ops.OPS.append(op)
    dve_ops._SUB_OPCODE_FOR_NAME[op.name] = (
        dve_ops._CUSTOM_DVE_ROW_BASE + len(dve_ops.OPS) - 1
    )
    dve_ops.CUSTOM_DVE_SPECS[op.name] = spec
    assert dve_ops._SUB_OPCODE_FOR_NAME[op.name] < 0x20
    return op


def _build():
    import concourse.tile as tile
    import concourse.mybir as mybir
    from concourse import bacc

    f16 = mybir.dt.float16
    f32 = mybir.dt.float32

    am_op = _register_addmax()

    nc = bacc.Bacc("TRN2", target_bir_lowering=False, debug=False)
    xe_t = nc.dram_tensor("xe", [128, ROWS, W + 2], f16, kind="ExternalInput")
    x2_t = nc.dram_tensor("x2", [128, ROWS, W], f16, kind="ExternalInput")
    k_t = nc.dram_tensor("k", [128, 11], f32, kind="ExternalInput")
    o_t = nc.dram_tensor("out", [128, HALF, W], f16, kind="ExternalOutput")

    def am(out, in0, k_col, in1):
        bi = nc.vector._custom_dve(
            am_op, out=out, in0=in0, in1=in1, s0=kb[:, k_col : k_col + 1]
        )
        bi.ins.perf_max = 1
        return bi

    RMAX = max(CHUNKS)
    starts = [sum(CHUNKS[:i]) for i in range(len(CHUNKS))]
    with tile.TileContext(nc) as tc:
        with (
            tc.tile_pool(name="const", bufs=1) as cpool,
            tc.tile_pool(name="xin", bufs=4) as xpool,
            tc.tile_pool(name="x2in", bufs=4) as x2pool,
            tc.tile_pool(name="o", bufs=3) as opool,
        ):
            # kb first on the sync HWDGE queue: every AM op reads it, and the
            # GpSimd SWDGE path would gate the first op by ~7us.
            kb = cpool.tile([128, 11], f32)
            nc.sync.dma_start(kb[:], k_t[:])

            def load_chunk(ci):
                R, r0 = CHUNKS[ci], starts[ci]
                xe = xpool.tile([128, RMAX + 2, W + 2], f16, tag="xe")
                x2 = x2pool.tile([128, RMAX + 2, W], f16, tag="x2")
                # Both streams on the sync queue, strictly in chunk order:
                # deterministic FIFO arrival, full ring rate per transfer.
                nc.sync.dma_start(xe[:, 0 : R + 2, :], xe_t[:, r0 : r0 + R + 2, :])
                nc.sync.dma_start(x2[:, 0 : R + 2, :], x2_t[:, r0 : r0 + R + 2, :])
                return xe, x2

            loads = [load_chunk(0), load_chunk(1), load_chunk(2)]
            for ci, R in enumerate(CHUNKS):
                r0 = starts[ci]
                nxt = ci + 1
                if ci + 3 < len(CHUNKS):
                    loads.append(load_chunk(ci + 3))
                xe, x2 = loads[ci]

                o = opool.tile([128, RMAX, W], f16, tag="o")
                # terms (dy+1, dx+1, k index): xe cols 0/2 + x2 rows; T4 seeds.
                am(o[:, 0:R, :], xe[:, 0:R, 0:W], 0, x2[:, 1 : R + 1, :])
                am(o[:, 0:R, :], xe[:, 0:R, 2 : W + 2], 2, o[:, 0:R, :])
                am(o[:, 0:R, :], xe[:, 1 : R + 1, 0:W], 3, o[:, 0:R, :])
                am(o[:, 0:R, :], xe[:, 1 : R + 1, 2 : W + 2], 5, o[:, 0:R, :])
                am(o[:, 0:R, :], xe[:, 2 : R + 2, 0:W], 6, o[:, 0:R, :])
                am(o[:, 0:R, :], xe[:, 2 : R + 2, 2 : W + 2], 8, o[:, 0:R, :])
                am(o[:, 0:R, :], x2[:, 0:R, :], 9, o[:, 0:R, :])
                am(o[:, 0:R, :], x2[:, 2 : R + 2, :], 10, o[:, 0:R, :])

                # Mid-chunk output DMAs on the (idle) GpSimd SWDGE queue; the
                # last chunk uses the lower-latency sync HWDGE queue.
                eng = nc.sync if nxt == len(CHUNKS) else nc.gpsimd
                eng.dma_start(o_t[:, r0 : r0 + R, :], o[:, 0:R, :])
    nc.finalize()
    return nc


LAST_RESULT = None


def kernel(x, kernel):
    """x: [8,64,224,224] f32; kernel: [1,64,9,1,1] f32 -> [8,64,224,224] f32."""
    global LAST_RESULT
    from concourse.bass_utils import run_bass_kernel_spmd

    if "nc" not in _CACHE:
        _CACHE["nc"] = _build()
    nc = _CACHE["nc"]

    B = x.shape[0]
    kf = np.ascontiguousarray(np.asarray(kernel, np.float32).reshape(C, 9))

    xp = np.zeros((B, C, H + 2, W + 2), np.float16)
    xp[:, :, 1 : H + 1, 1 : W + 1] = x
    # xe: [B, 128, 114, 226], partition p = half*64 + c
    xe = np.concatenate(
        [xp[:, :, 0:ROWS, :], xp[:, :, HALF : HALF + ROWS, :]], axis=1
    )
    # x2 = xpad(col +1) + k4 (fp32 add, fp16 round) -> the three dx=0 terms
    x2full = (
        np.float32(xp[:, :, :, 1 : W + 1]) + kf[None, :, 4, None, None]
    ).astype(np.float16)
    x2 = np.concatenate(
        [x2full[:, :, 0:ROWS, :], x2full[:, :, HALF : HALF + ROWS, :]], axis=1
    )
    # kb cols 0..8 = k0..k8; col 9 = k1-k4; col 10 = k7-k4 (x2 deltas)
    kb = np.concatenate(
        [kf, (kf[:, 1] - kf[:, 4])[:, None], (kf[:, 7] - kf[:, 4])[:, None]], axis=1
    )
    kb = np.concatenate([kb, kb], axis=0)  # [128, 11]

    in_maps = [{"xe": xe[b], "x2": x2[b], "k": kb} for b in range(B)]
    res = run_bass_kernel_spmd(nc, in_maps, core_ids=list(range(B)))
    LAST_RESULT = res
    out = np.stack([r["out"] for r in res.results], axis=0)  # [B, 128, 112, 224]
    out = out.reshape(B, 2, C, HALF, W).transpose(0, 2, 1, 3, 4).reshape(B, C, H, W)
    return out.astype(np.float32)



# revision 4
# speedup vs baseline: 1.2829x; 1.2829x over previous
"""Morphological dilation (depthwise 3x3, additive SE) on 8 TRN2 NeuronCores.

out[b,c,h,w] = max_{dy,dx in {-1,0,1}} ( x[b,c,h+dy,w+dx] + k[c, (dy+1)*3+(dx+1)] )
with zero padding outside the image.

Sharding: batch -> 8 cores (1 image each). Per core, partitions = (h_half, c)
(2*64 = 128), free dim = flat (row-major, 226-wide padded rows).

The whole 9-term reduction runs as THREE custom-DVE passes per tile (T3A, a
hand-authored 1x-mode 8-block uop program):

  T3A: out[j] = max(in0[j-2]+s0, in0[j-1]+kL, in0[j]+s1, in1[1+j])

i.e. one pass folds a full window ROW (3 horizontal taps) into the running
max. The two off-alignment taps come from 1-beat swap-flop delay elements
(BYPASS+swap = emit previous beat's value, capture this beat's); the third
per-channel constant kL is latched from in1's first element by an init uop
(the C3-spill pattern). Streams are flat [R,226] runs, so row boundaries
need no AP tricks - the 2 pad columns absorb window wrap, and each output
buffer keeps [latch][2 garbage] slots ahead of its data.

Pass structure per chunk (output rows r0..r0+R):
  P1: o1 = row r0   taps (k0,k1,k2) + dummy(-6e4) acc seed
  P2: o2 = row r0+1 taps (k3,k4,k5) + o1
  P3: o3 = row r0+2 taps (k6,k7,k8) + o2   -> DMA out

vs. the previous 8-pass 2x_1p add-max chain: 3 passes x 1 elem/cyc beats
8 passes x 2 elem/cyc, and the x2 (host-preshifted) input stream is gone -
HBM traffic drops from ~19.6MB to ~13.1MB per core. All DMas ride the sync
HWDGE queue: T3A reads both DVE ports every cycle, and one of them is
shared with GpSimd, so SWDGE (gpsimd) descriptor generation would contend.
"""

import numpy as np

_CACHE = {}

C = 64
H = 224
W = 224
WF = W + 2  # padded row width
HALF = 112
ROWS = HALF + 2  # per-half rows incl. 1-row halo each side
CHUNKS = (12, 24, 28, 28, 20)
RMAX = max(CHUNKS)
BUF = 3 + RMAX * WF  # [latch][2 garbage][R*226 data]
NEG = -60000.0  # neutral acc seed (fp16-safe, dominates any |x+k| <= ~8)

_T3A_NAME = "T3A_WINMAX_ANT"


def _register_t3a():
    """Register the fused 3-tap window max-plus custom DVE op (idempotent).

    1x mode, 8 blocks, 2 uops:
      uop0 (init): consume in1[0], latch into blk4's swap flop (kL).
      uop1 (steady): out[j] = max(x[j-2]+C0, x[j-1]+kL, x[j]+C1, acc[j])
    with x = in0 (SRC_0), acc = in1[1:] (SRC_1). The 1- and 2-beat lookbacks
    are BYPASS+swap stages: ALU emits CURR_SWAP_OUT (previous beat's capture)
    while the swap flop latches the complementary operand (this beat's value).
    out[0], out[1] are stale-flop garbage; callers land them in pad slots.
    """
    from concourse import dve_ops
    from concourse.dve_spec import Spec, Src0, Src1, C0, maxx
    from concourse.dve_uop import (
        AluInp,
        AluOp,
        DelayInp,
        DveOpSpec,
        ENABLE,
        InpSel,
        OutPath,
        OutSel,
        Trigger,
        UopConfig,
    )

    if _T3A_NAME in dve_ops._SUB_OPCODE_FOR_NAME:
        return next(op for op in dve_ops.OPS if op.name == _T3A_NAME)

    def _ref(in0, in1, s0, s1, imm2):
        x = np.asarray(in0, np.float32)
        P = x.shape[0]
        kl = np.asarray(in1[:, 0:1], np.float32)
        acc = np.asarray(in1[:, 1:], np.float32)
        s0 = np.asarray(s0, np.float32).reshape(P, 1)
        s1 = np.asarray(s1, np.float32).reshape(P, 1)
        ninf = np.float32(-1e30)
        xm1 = np.concatenate([np.full((P, 1), ninf, np.float32), x[:, :-1]], 1)
        xm2 = np.concatenate([np.full((P, 2), ninf, np.float32), x[:, :-2]], 1)
        return np.maximum.reduce([xm2 + s0, xm1 + kl, x + s1, acc])

    # Body is metadata only (rd1_en / no-C2 checks); the uops are the truth.
    spec = Spec(body=maxx(Src0 + C0, Src1), reference=_ref)

    def _build_uops():
        u0 = UopConfig()
        u0.enable_input(InpSel.SRC_1, 1)
        u0.require_inp1 = 1
        u0.trigger = (Trigger.COUNT, Trigger.NONE, Trigger.NONE)
        u0.repeat_count = 1
        u0.next_uop = (1, 0, 0)
        dp = u0.datapath_config
        for b in range(4):
            dp[b].pass_through_delay(0)  # chain0 <- lane1 = SRC_1 (kL)
        dp[4].enable_alu(AluOp.BYPASS, AluInp.PREV_DELAY_0, AluInp.PREV_DELAY_0)
        dp[4].swap_enable = ENABLE  # BYPASS(a): swap <- b = kL

        # lanes: 0=SRC_0(x), 1=SRC_1(acc), 2=CONST_0(s0), 3=CONST_1(s1)
        u1 = UopConfig()
        u1.enable_input(InpSel.SRC_0, 0)
        u1.enable_input(InpSel.SRC_1, 1)
        u1.enable_input(InpSel.CONST_0, 2)
        u1.enable_input(InpSel.CONST_1, 3)
        u1.require_inp0 = 1
        u1.require_inp1 = 1
        u1.trigger = (Trigger.SRC_TENSOR_DONE, Trigger.NONE, Trigger.NONE)
        dp = u1.datapath_config
        # blk0: t2 = x + C1; chains: 0<-acc, 1<-C0, 2<-raw x
        dp[0].enable_alu(AluOp.ADD, AluInp.PREV_ALU_OUT, AluInp.PREV_DELAY_2)
        dp[0].pass_through_delay(0, 1)
        dp[0].enable_delay_from_src(DelayInp.PREV_ALU_OUT, 2)
        # blk1: 1-beat delay: out = x(j-1), swap <- x(j)
        dp[1].enable_alu(AluOp.BYPASS, AluInp.CURR_SWAP_OUT, AluInp.PREV_DELAY_2)
        dp[1].swap_enable = ENABLE
        dp[1].pass_through_delay(0, 1)
        dp[1].enable_delay_from_src(DelayInp.PREV_ALU_OUT, 3)  # chain3 <- t2
        # blk2: 1-beat delay: out = x(j-2), swap <- x(j-1)
        dp[2].enable_alu(AluOp.BYPASS, AluInp.CURR_SWAP_OUT, AluInp.PREV_ALU_OUT)
        dp[2].swap_enable = ENABLE
        dp[2].pass_through_delay(0, 1, 3)
        dp[2].enable_delay_from_src(DelayInp.PREV_ALU_OUT, 4)  # chain4 <- x(j-1)
        # blk3: t0 = x(j-2) + C0
        dp[3].enable_alu(AluOp.ADD, AluInp.PREV_ALU_OUT, AluInp.PREV_DELAY_1)
        dp[3].pass_through_delay(0, 3, 4)
        # blk4: tmid = x(j-1) + kL (kL persists in this blk's swap flop)
        dp[4].enable_alu(AluOp.ADD, AluInp.PREV_DELAY_4, AluInp.CURR_SWAP_OUT)
        dp[4].pass_through_delay(0, 3)
        dp[4].enable_delay_from_src(DelayInp.PREV_ALU_OUT, 1)  # chain1 <- t0
        # blk5: m1 = max(tmid, t0)
        dp[5].enable_alu(AluOp.MAX, AluInp.PREV_ALU_OUT, AluInp.PREV_DELAY_1)
        dp[5].pass_through_delay(0, 3)
        # blk6: m2 = max(m1, t2)
        dp[6].enable_alu(AluOp.MAX, AluInp.PREV_ALU_OUT, AluInp.PREV_DELAY_3)
        dp[6].pass_through_delay(0)
        # blk7: out = max(m2, acc)
        dp[7].enable_alu(AluOp.MAX, AluInp.PREV_ALU_OUT, AluInp.PREV_DELAY_0)
        u1.enable_output(OutSel.ALU_OUT, OutPath.WR0_LO)
        return [u0, u1]

    class _T3AOp:
        name = _T3A_NAME
        subdim = False
        perf_en = {}
        uops_sha = {}

        def __init__(self):
            self.spec = spec
            self._cache = {}

        def compile(self, ver):
            if ver in self._cache:
                return self._cache[ver]
            assert ver == "v3", "T3A authored for TRN2/v3"
            s = DveOpSpec(
                name=self.name,
                opcode=dve_ops.get_dve_sub_opcode(self.name),
                uops=_build_uops(),
                rd1_en=True,
                perf_max=0,
            )
            s.validate(ver)
            self._cache[ver] = s
            return s

    op = _T3AOp()
    dve_ops.OPS.append(op)
    dve_ops._SUB_OPCODE_FOR_NAME[op.name] = (
        dve_ops._CUSTOM_DVE_ROW_BASE + len(dve_ops.OPS) - 1
    )
    dve_ops.CUSTOM_DVE_SPECS[op.name] = spec
    assert dve_ops._SUB_OPCODE_FOR_NAME[op.name] < 0x20
    return op


def _build():
    import concourse.tile as tile
    import concourse.mybir as mybir
    from concourse import bacc

    f16 = mybir.dt.float16
    f32 = mybir.dt.float32

    t3a = _register_t3a()

    nc = bacc.Bacc("TRN2", target_bir_lowering=False, debug=False)
    xe_t = nc.dram_tensor("xe", [128, ROWS * WF + 2], f16, kind="ExternalInput")
    k_t = nc.dram_tensor("k", [128, 9], f32, kind="ExternalInput")
    o_t = nc.dram_tensor("out", [128, HALF * WF], f16, kind="ExternalOutput")

    starts = [sum(CHUNKS[:i]) for i in range(len(CHUNKS))]
    with tile.TileContext(nc) as tc:
        with (
            tc.tile_pool(name="const", bufs=1) as cpool,
            tc.tile_pool(name="xin", bufs=3) as xpool,
            tc.tile_pool(name="o", bufs=3) as opool,
        ):
            kb = cpool.tile([128, 9], f32)
            nc.sync.dma_start(kb[:], k_t[:])
            dummy = cpool.tile([128, BUF], f16)
            o1 = cpool.tile([128, BUF], f16)
            o2 = cpool.tile([128, BUF], f16)
            # Latch slots: dummy[0]=k1, o1[0]=k4, o2[0]=k7 (written once; the
            # passes only ever write cols 1.., so the slots persist). DVE
            # copies, not tiny DMAs: 2-byte-per-partition DMas proved flaky
            # (one partition on one core read a stale latch). The -6e4
            # acc-seed fill runs on the otherwise idle GpSimd first.
            nc.gpsimd.memset(dummy[:, 0:BUF], NEG)
            nc.vector.tensor_copy(dummy[:, 0:1], kb[:, 1:2])
            nc.vector.tensor_copy(o1[:, 0:1], kb[:, 4:5])
            nc.vector.tensor_copy(o2[:, 0:1], kb[:, 7:8])

            def t3(out, in0, in1, s0c, s1c):
                nc.vector._custom_dve(
                    t3a, out=out, in0=in0, in1=in1,
                    s0=kb[:, s0c : s0c + 1], s1=kb[:, s1c : s1c + 1],
                )

            def load_chunk(ci):
                R, r0 = CHUNKS[ci], starts[ci]
                xe = xpool.tile([128, (RMAX + 2) * WF + 2], f16, tag="xe")
                n = (R + 2) * WF + 2
                nc.sync.dma_start(xe[:, 0:n], xe_t[:, r0 * WF : r0 * WF + n])
                return xe

            loads = [load_chunk(0), load_chunk(1), load_chunk(2)]
            for ci, R in enumerate(CHUNKS):
                r0 = starts[ci]
                if ci + 3 < len(CHUNKS):
                    loads.append(load_chunk(ci + 3))
                xe = loads[ci]
                o3 = opool.tile([128, BUF], f16, tag="o")
                N = 2 + R * WF
                t3(o1[:, 1 : 1 + N], xe[:, 0:N], dummy[:, 0 : N + 1], 0, 2)
                t3(o2[:, 1 : 1 + N], xe[:, WF : WF + N], o1[:, 0 : N + 1], 3, 5)
                t3(o3[:, 1 : 1 + N], xe[:, 2 * WF : 2 * WF + N], o2[:, 0 : N + 1], 6, 8)
                nc.sync.dma_start(
                    o_t[:, r0 * WF : (r0 + R) * WF], o3[:, 3 : 3 + R * WF]
                )
    nc.finalize()
    return nc


LAST_RESULT = None


def kernel(x, kernel):
    """x: [8,64,224,224] f32; kernel: [1,64,9,1,1] f32 -> [8,64,224,224] f32."""
    global LAST_RESULT
    from concourse.bass_utils import run_bass_kernel_spmd

    if "nc" not in _CACHE:
        _CACHE["nc"] = _build()
    nc = _CACHE["nc"]

    B = x.shape[0]
    kf = np.ascontiguousarray(np.asarray(kernel, np.float32).reshape(C, 9))
    kb = np.concatenate([kf, kf], axis=0)  # [128, 9], partition p = half*64+c

    xp = np.zeros((B, C, H + 2, W + 2), np.float16)
    xp[:, :, 1 : H + 1, 1 : W + 1] = x
    # xe: [B, 128, 114*226+2] flat, partition p = half*64 + c
    xe3 = np.concatenate(
        [xp[:, :, 0:ROWS, :], xp[:, :, HALF : HALF + ROWS, :]], axis=1
    ).reshape(B, 128, ROWS * WF)
    xe = np.zeros((B, 128, ROWS * WF + 2), np.float16)
    xe[:, :, : ROWS * WF] = xe3
    # aux: latch consts [k1, k4, k7] per partition, fp16
    aux = np.ascontiguousarray(kb[:, [1, 4, 7]].astype(np.float16))

    in_maps = [{"xe": xe[b], "k": kb, "aux": aux} for b in range(B)]
    res = run_bass_kernel_spmd(nc, in_maps, core_ids=list(range(B)))
    LAST_RESULT = res
    out = np.stack([r["out"] for r in res.results], axis=0)  # [B, 128, 112*226]
    out = out.reshape(B, 2, C, HALF, WF)[:, :, :, :, 0:W]
    out = out.transpose(0, 2, 1, 3, 4).reshape(B, C, H, W)
    return out.astype(np.float32)


# revision 10
# speedup vs baseline: 1.3175x; 1.0269x over previous
"""Morphological dilation (depthwise 3x3, additive SE) on 8 TRN2 NeuronCores.

out[b,c,h,w] = max_{dy,dx in {-1,0,1}} ( x[b,c,h+dy,w+dx] + k[c, (dy+1)*3+(dx+1)] )
with zero padding outside the image.

Sharding: batch -> 8 cores (1 image each). Per core, partitions = (h_half, c)
(2*64 = 128), free dim = flat (row-major, 226-wide padded rows).

The whole 9-term reduction runs as THREE custom-DVE passes per tile (T3A, a
hand-authored 1x-mode 8-block uop program):

  T3A: out[j] = max(in0[j-2]+s0, in0[j-1]+kL, in0[j]+s1, in1[1+j])

i.e. one pass folds a full window ROW (3 horizontal taps) into the running
max. The two off-alignment taps come from 1-beat swap-flop delay elements
(BYPASS+swap = emit previous beat's value, capture this beat's); the third
per-channel constant kL is latched from in1's first element by an init uop
(the C3-spill pattern). Streams are flat [R,226] runs, so row boundaries
need no AP tricks - the 2 pad columns absorb window wrap, and each output
buffer keeps [latch][2 garbage] slots ahead of its data.

Pass structure per chunk (output rows r0..r0+R):
  P1: o1 = row r0   taps (k0,k1,k2)        [T3N: no accumulator]
  P2: o2 = row r0+1 taps (k3,k4,k5) + o1   [T3A]
  P3: o3 = row r0+2 taps (k6,k7,k8) + o2   [T3A] -> DMA out

vs. the previous 8-pass 2x_1p add-max chain: 3 passes x 1 elem/cyc beats
8 passes x 2 elem/cyc, and the x2 (host-preshifted) input stream is gone -
HBM traffic drops from ~19.6MB to ~13.1MB per core. All DMas ride the sync
HWDGE queue: T3A reads both DVE ports every cycle, and one of them is
shared with GpSimd, so SWDGE (gpsimd) descriptor generation would contend.
"""

import numpy as np

_CACHE = {}

C = 64
H = 224
W = 224
WF = W + 2  # padded row width
HALF = 112
ROWS = HALF + 2  # per-half rows incl. 1-row halo each side
CHUNKS = (8, 48, 48, 8)
RMAX = max(CHUNKS)
BUF = 3 + RMAX * WF  # [latch][2 garbage][R*226 data]

_T3A_NAME = "T3A_WINMAX_ANT"
_T3N_NAME = "T3N_WINMAX_ANT"


def _register_winmax(name, with_acc):
    """Register a fused 3-tap window max-plus custom DVE op (idempotent).

    1x mode, 8 blocks, 2 uops:
      uop0 (init): consume in1[0], latch into blk4's swap flop (kL).
      uop1 (steady):
        T3A (with_acc): out[j] = max(x[j-2]+C0, x[j-1]+kL, x[j]+C1, acc[j])
                        with acc = in1[1:]
        T3N (no acc):   out[j] = max(x[j-2]+C0, x[j-1]+kL, x[j]+C1)
                        in1 = [P,1], just the latch element
    x = in0 (SRC_0). The 1- and 2-beat lookbacks are BYPASS+swap stages: the
    ALU emits CURR_SWAP_OUT (previous beat's capture) while the swap flop
    latches the complementary operand (this beat's value). out[0], out[1]
    are stale-flop garbage; callers land them in pad slots.
    """
    from concourse import dve_ops
    from concourse.dve_spec import Spec, Src0, Src1, C0, maxx
    from concourse.dve_uop import (
        AluInp,
        AluOp,
        DelayInp,
        DveOpSpec,
        ENABLE,
        InpSel,
        OutPath,
        OutSel,
        Trigger,
        UopConfig,
    )

    if name in dve_ops._SUB_OPCODE_FOR_NAME:
        return next(op for op in dve_ops.OPS if op.name == name)

    def _ref(in0, in1, s0, s1, imm2):
        x = np.asarray(in0, np.float32)
        P = x.shape[0]
        kl = np.asarray(in1[:, 0:1], np.float32)
        s0 = np.asarray(s0, np.float32).reshape(P, 1)
        s1 = np.asarray(s1, np.float32).reshape(P, 1)
        ninf = np.float32(-1e30)
        xm1 = np.concatenate([np.full((P, 1), ninf, np.float32), x[:, :-1]], 1)
        xm2 = np.concatenate([np.full((P, 2), ninf, np.float32), x[:, :-2]], 1)
        terms = [xm2 + s0, xm1 + kl, x + s1]
        if with_acc:
            terms.append(np.asarray(in1[:, 1:], np.float32))
        return np.maximum.reduce(terms)

    # Body is metadata only (rd1_en / no-C2 checks); the uops are the truth.
    spec = Spec(body=maxx(Src0 + C0, Src1), reference=_ref)

    def _build_uops():
        u0 = UopConfig()
        u0.enable_input(InpSel.SRC_1, 1)
        u0.require_inp1 = 1
        u0.trigger = (Trigger.COUNT, Trigger.NONE, Trigger.NONE)
        u0.repeat_count = 1
        u0.next_uop = (1, 0, 0)
        dp = u0.datapath_config
        for b in range(4):
            dp[b].pass_through_delay(0)  # chain0 <- lane1 = SRC_1 (kL)
        dp[4].enable_alu(AluOp.BYPASS, AluInp.PREV_DELAY_0, AluInp.PREV_DELAY_0)
        dp[4].swap_enable = ENABLE  # BYPASS(a): swap <- b = kL

        # lanes: 0=SRC_0(x), 1=SRC_1(acc), 2=CONST_0(s0), 3=CONST_1(s1)
        u1 = UopConfig()
        u1.enable_input(InpSel.SRC_0, 0)
        u1.enable_input(InpSel.CONST_0, 2)
        u1.enable_input(InpSel.CONST_1, 3)
        u1.require_inp0 = 1
        if with_acc:
            u1.enable_input(InpSel.SRC_1, 1)
            u1.require_inp1 = 1
        u1.trigger = (Trigger.SRC_TENSOR_DONE, Trigger.NONE, Trigger.NONE)
        dp = u1.datapath_config
        acc_chain = (0,) if with_acc else ()
        # blk0: t2 = x + C1; chains: [0<-acc,] 1<-C0, 2<-raw x
        dp[0].enable_alu(AluOp.ADD, AluInp.PREV_ALU_OUT, AluInp.PREV_DELAY_2)
        dp[0].pass_through_delay(*acc_chain, 1)
        dp[0].enable_delay_from_src(DelayInp.PREV_ALU_OUT, 2)
        # blk1: 1-beat delay: out = x(j-1), swap <- x(j)
        dp[1].enable_alu(AluOp.BYPASS, AluInp.CURR_SWAP_OUT, AluInp.PREV_DELAY_2)
        dp[1].swap_enable = ENABLE
        dp[1].pass_through_delay(*acc_chain, 1)
        dp[1].enable_delay_from_src(DelayInp.PREV_ALU_OUT, 3)  # chain3 <- t2
        # blk2: 1-beat delay: out = x(j-2), swap <- x(j-1)
        dp[2].enable_alu(AluOp.BYPASS, AluInp.CURR_SWAP_OUT, AluInp.PREV_ALU_OUT)
        dp[2].swap_enable = ENABLE
        dp[2].pass_through_delay(*acc_chain, 1, 3)
        dp[2].enable_delay_from_src(DelayInp.PREV_ALU_OUT, 4)  # chain4 <- x(j-1)
        # blk3: t0 = x(j-2) + C0
        dp[3].enable_alu(AluOp.ADD, AluInp.PREV_ALU_OUT, AluInp.PREV_DELAY_1)
        dp[3].pass_through_delay(*acc_chain, 3, 4)
        # blk4: tmid = x(j-1) + kL (kL persists in this blk's swap flop)
        dp[4].enable_alu(AluOp.ADD, AluInp.PREV_DELAY_4, AluInp.CURR_SWAP_OUT)
        dp[4].pass_through_delay(*acc_chain, 3)
        dp[4].enable_delay_from_src(DelayInp.PREV_ALU_OUT, 1)  # chain1 <- t0
        # blk5: m1 = max(tmid, t0)
        dp[5].enable_alu(AluOp.MAX, AluInp.PREV_ALU_OUT, AluInp.PREV_DELAY_1)
        dp[5].pass_through_delay(*acc_chain, 3)
        # blk6: m2 = max(m1, t2)
        dp[6].enable_alu(AluOp.MAX, AluInp.PREV_ALU_OUT, AluInp.PREV_DELAY_3)
        dp[6].pass_through_delay(*acc_chain)
        # blk7: out = max(m2, acc) / pass-through m2
        if with_acc:
            dp[7].enable_alu(AluOp.MAX, AluInp.PREV_ALU_OUT, AluInp.PREV_DELAY_0)
        else:
            dp[7].pass_through_alu()
        u1.enable_output(OutSel.ALU_OUT, OutPath.WR0_LO)
        return [u0, u1]

    class _WinMaxOp:
        subdim = False
        perf_en = {}
        uops_sha = {}

        def __init__(self):
            self.name = name
            self.spec = spec
            self._cache = {}

        def compile(self, ver):
            if ver in self._cache:
                return self._cache[ver]
            assert ver == "v3", "winmax ops authored for TRN2/v3"
            s = DveOpSpec(
                name=self.name,
                opcode=dve_ops.get_dve_sub_opcode(self.name),
                uops=_build_uops(),
                rd1_en=True,
                perf_max=0,
            )
            s.validate(ver)
            self._cache[ver] = s
            return s

    op = _WinMaxOp()
    dve_ops.OPS.append(op)
    dve_ops._SUB_OPCODE_FOR_NAME[op.name] = (
        dve_ops._CUSTOM_DVE_ROW_BASE + len(dve_ops.OPS) - 1
    )
    dve_ops.CUSTOM_DVE_SPECS[op.name] = spec
    assert dve_ops._SUB_OPCODE_FOR_NAME[op.name] < 0x20
    return op


def _build():
    import concourse.tile as tile
    import concourse.mybir as mybir
    from concourse import bacc

    f16 = mybir.dt.float16
    f32 = mybir.dt.float32

    t3a = _register_winmax(_T3A_NAME, with_acc=True)
    t3n = _register_winmax(_T3N_NAME, with_acc=False)

    nc = bacc.Bacc("TRN2", target_bir_lowering=False, debug=False)
    xe_t = nc.dram_tensor("xe", [128, ROWS * WF + 2], f16, kind="ExternalInput")
    k_t = nc.dram_tensor("k", [128, 9], f32, kind="ExternalInput")
    o_t = nc.dram_tensor("out", [128, HALF * WF], f16, kind="ExternalOutput")

    starts = [sum(CHUNKS[:i]) for i in range(len(CHUNKS))]
    with tile.TileContext(nc) as tc:
        with (
            tc.tile_pool(name="const", bufs=1) as cpool,
            tc.tile_pool(name="xin", bufs=3) as xpool,
            tc.tile_pool(name="o", bufs=3) as opool,
        ):
            kb = cpool.tile([128, 9], f32)
            nc.sync.dma_start(kb[:], k_t[:])
            kaux = cpool.tile([128, 1], f16)
            o1 = cpool.tile([128, BUF], f16)
            o2 = cpool.tile([128, BUF], f16)
            # Latch slots: kaux=k1 (P1's in1), o1[0]=k4, o2[0]=k7 (written
            # once; the passes only ever write cols 1.., so the slots
            # persist). DVE copies, not tiny DMAs: 2-byte-per-partition DMAs
            # proved flaky (one partition on one core read a stale latch).
            nc.vector.tensor_copy(kaux[:], kb[:, 1:2])
            nc.vector.tensor_copy(o1[:, 0:1], kb[:, 4:5])
            nc.vector.tensor_copy(o2[:, 0:1], kb[:, 7:8])

            def t3(op, out, in0, in1, s0c, s1c):
                nc.vector._custom_dve(
                    op, out=out, in0=in0, in1=in1,
                    s0=kb[:, s0c : s0c + 1], s1=kb[:, s1c : s1c + 1],
                )

            def load_chunk(ci):
                R, r0 = CHUNKS[ci], starts[ci]
                xe = xpool.tile([128, (RMAX + 2) * WF + 2], f16, tag="xe")
                n = (R + 2) * WF + 2
                nc.sync.dma_start(xe[:, 0:n], xe_t[:, r0 * WF : r0 * WF + n])
                return xe

            loads = [load_chunk(0), load_chunk(1), load_chunk(2)]
            for ci, R in enumerate(CHUNKS):
                r0 = starts[ci]
                if ci + 3 < len(CHUNKS):
                    loads.append(load_chunk(ci + 3))
                xe = loads[ci]
                o3 = opool.tile([128, BUF], f16, tag="o")
                N = 2 + R * WF
                t3(t3n, o1[:, 1 : 1 + N], xe[:, 0:N], kaux[:], 0, 2)
                t3(t3a, o2[:, 1 : 1 + N], xe[:, WF : WF + N], o1[:, 0 : N + 1], 3, 5)
                t3(t3a, o3[:, 1 : 1 + N], xe[:, 2 * WF : 2 * WF + N], o2[:, 0 : N + 1], 6, 8)
                nc.sync.dma_start(
                    o_t[:, r0 * WF : (r0 + R) * WF], o3[:, 3 : 3 + R * WF]
                )
    nc.finalize()
    return nc


LAST_RESULT = None


def kernel(x, kernel):
    """x: [8,64,224,224] f32; kernel: [1,64,9,1,1] f32 -> [8,64,224,224] f32."""
    global LAST_RESULT
    from concourse.bass_utils import run_bass_kernel_spmd

    if "nc" not in _CACHE:
        _CACHE["nc"] = _build()
    nc = _CACHE["nc"]

    B = x.shape[0]
    kf = np.ascontiguousarray(np.asarray(kernel, np.float32).reshape(C, 9))
    kb = np.concatenate([kf, kf], axis=0)  # [128, 9], partition p = half*64+c

    xp = np.zeros((B, C, H + 2, W + 2), np.float16)
    xp[:, :, 1 : H + 1, 1 : W + 1] = x
    # xe: [B, 128, 114*226+2] flat, partition p = half*64 + c
    xe3 = np.concatenate(
        [xp[:, :, 0:ROWS, :], xp[:, :, HALF : HALF + ROWS, :]], axis=1
    ).reshape(B, 128, ROWS * WF)
    xe = np.zeros((B, 128, ROWS * WF + 2), np.float16)
    xe[:, :, : ROWS * WF] = xe3
    # aux: latch consts [k1, k4, k7] per partition, fp16
    aux = np.ascontiguousarray(kb[:, [1, 4, 7]].astype(np.float16))

    in_maps = [{"xe": xe[b], "k": kb, "aux": aux} for b in range(B)]
    res = run_bass_kernel_spmd(nc, in_maps, core_ids=list(range(B)))
    LAST_RESULT = res
    out = np.stack([r["out"] for r in res.results], axis=0)  # [B, 128, 112*226]
    out = out.reshape(B, 2, C, HALF, WF)[:, :, :, :, 0:W]
    out = out.transpose(0, 2, 1, 3, 4).reshape(B, C, H, W)
    return out.astype(np.float32)
